# revision 27
# baseline (speedup 1.0000x reference)
"""DKVMN scatter_memory kernel for 8 Trainium2 NeuronCores.

Math: the reference scan only ever uses the (B, M, Dv) memory through
read @ Wf_r, so the whole recurrence collapses to a 32-dim linear
cumulative sum:

  S  = softmax(Eq @ Wa + ba)            (100 x 32)  per-vocab att rows
  cq = Eq @ Wf[:64] + bf                (100,)
  cv = Ev @ Wf[64:]                     (100,)
  w  = (2q + a) % 100
  pred[t,b] = cq[q[t,b]] + sum_{s<t} cv[w[s,b]] * <S[q[t,b]], S[q[s,b]]>

Per core (batch-sharded, Bs=128): the host precomputes a 120-row fp8
index encoding per token (pure index preprocessing; 0/1 exact in fp8):
rows 0:100 one-hot(q), rows 100:110 one-hot(w%10), rows 110:120
one-hot(w//10).  One 54-col matmul per batch element against a packed
table mcat = [S | cvt | ind | cq | pad] gathers the S-row, cq, and the
digit factors of cv[w] (cv[w] = sum_j 1{hi=j} * cv[10j+lo]).  The
cumsum over t is a strict-upper-triangular matmul.  Layout: t on
partitions; A/V/C/A*C are stored M-MAJOR ([m, b] in the free dim) so
the cv broadcast is outer-stride-0 (keeps DVE 2x mode), the A*C
product is flat 2D, and the reduce over m is a log-fold of flat adds.
All elementwise work is on DVE (GpSimd contends with DVE for the SBUF
port); Scalar does the PSUM->SBUF extractions.
"""
import functools
import numpy as np
import ml_dtypes

import concourse.bass as bass
import concourse.bacc as bacc
import concourse.mybir as mybir
from concourse import tile
from concourse.bass_utils import run_bass_kernel_spmd

T, B, M, DQ, DV, VOCAB = 128, 1024, 32, 64, 64, 100
NCORES = 8
BS = B // NCORES  # 128
N = T * BS        # tokens per core = 16384
R = 120           # one-hot rows: 100 q + 10 w-lo + 10 w-hi
GB = 32           # b per pass
NPASS = BS // GB  # 4
F32 = mybir.dt.float32
F16 = mybir.dt.float16
FP8 = mybir.dt.float8e4
AX = mybir.AxisListType
OP = mybir.AluOpType

# mcat column layout (53 used cols at stride 54):
#   0:32  S row      32:42 cvt (cv candidates given lo digit)
#   42:52 ind (1{hi=j})   52 cq   53 pad(0)
MC = 54

# pack_a (f16 [64, PCA]): matmul params, lands first
_EQT, _EVT, _WAQ, _WFR = 0, 100, 200, 233
PCA = 234
# pack_b (f16 [128, PCB]): bias row, ones row, strict-upper US
_BIA, _ONE, _US = 0, 34, 134
PCB = 262

# merged per-pass workspace (f16), column offsets
_A = 0          # [32m, 32b] gathered S rows, m-major
_DGC = 1024     # [22c, 32b] digit block, c-major (cvt 0:10 | ind | cq @20)
_CVP = 1728     # [10c, 32b] cvt * ind, c-major
_CVW = 2048     # [32b]      cv[w]
_V = 2080       # [32m, 32b] A * cv[w], m-major
_CG = 3104      # [16m, 32b] C for m 0:16 (f16; m 16:32 read from PSUM)
_AP = 3616      # [32m, 32b] A * C, m-major
_F1 = 4640      # [16m, 32b] fold scratch
_F2 = 5152      # [8m, 32b]
_F3 = 5408      # [4m, 32b]
_F4 = 5536      # [2m, 32b]
_O16 = 5600     # [32b]      reduced pred terms
WS = 5632


def _build():
    nc = bacc.Bacc("TRN2", num_devices=NCORES, debug=False, target_bir_lowering=False)
    d = {}
    d["pack_a"] = nc.dram_tensor("pack_a", [DQ, PCA], F16, kind="ExternalInput").ap()
    d["pack_b"] = nc.dram_tensor("pack_b", [128, PCB], F16, kind="ExternalInput").ap()
    d["skel"] = nc.dram_tensor("skel", [R, MC], F16, kind="ExternalInput").ap()
    # one-hot, stored so each pass-chunk [R, GB*T] is a contiguous block
    d["ohall"] = nc.dram_tensor("ohall", [NPASS * R, GB * T], FP8,
                                kind="ExternalInput").ap()
    preds = nc.dram_tensor("preds", [T, BS], F32, kind="ExternalOutput").ap()

    with tile.TileContext(nc) as tc:
        with (
            tc.tile_pool(name="sb", bufs=1) as sb,
            tc.tile_pool(name="oh", bufs=2) as ohp,
            tc.tile_pool(name="wk", bufs=4) as wk,
            tc.tile_pool(name="ps", bufs=2, space="PSUM") as ps,
        ):
            Pa = sb.tile([DQ, PCA], F16)
            Pb = sb.tile([128, PCB], F16)
            mcat = sb.tile([R, MC], F16)
            # pack_a first on the sync ring: the scalar ring is blocked by
            # the ACT table load early on, and pack_a gates the param chain
            nc.sync.dma_start(Pa[:], d["pack_a"][:])
            nc.scalar.dma_start(Pb[:], d["pack_b"][:])
            nc.gpsimd.dma_start(mcat[:], d["skel"][:])

            # one-hot chunks in pass order over three DMA streams (sync +
            # scalar HWDGE rings, gpsimd SWDGE), all contiguous in DRAM;
            # pool rotation (bufs=2) gates chunk i+2 on pass i.
            oh_t = []
            for ci in range(NPASS):
                t_ = ohp.tile([R, GB * T], FP8, tag="oh", name=f"oh_{ci}")
                r0 = ci * R
                nc.sync.dma_start(t_[0:40, :], d["ohall"][r0:r0 + 40, :])
                nc.scalar.dma_start(t_[40:80, :], d["ohall"][r0 + 40:r0 + 80, :])
                nc.gpsimd.dma_start(t_[80:R, :], d["ohall"][r0 + 80:r0 + R, :])
                oh_t.append(t_)

            us_t = Pb[:, _US:_US + 128]

            # ---- parameter tables (no device transposes) ----
            # cv row first: it feeds the mcat spray DMA (longest dep chain)
            p_cvr = ps.tile([1, VOCAB], F32, tag="pP2", bufs=2)
            nc.tensor.matmul(p_cvr[:], Pa[0:DV, _WFR:_WFR + 1],
                             Pa[0:DV, _EVT:_EVT + VOCAB], start=True, stop=True)
            cv_row = sb.tile([1, VOCAB], F16)
            nc.scalar.copy(cv_row[:], p_cvr[:])
            # Ev arrives row-permuted (perm(k) = 10(k%10) + k//10), so the cv
            # row comes out as cv_row[0, 10i+j] = cv[10j+i]; a plain [1,100]
            # -> [10,10] DMA spray then yields mcat[100+i, 32+j] = cv[10j+i].
            nc.scalar.dma_start(mcat[100:110, 32:42], cv_row[0:1, 0:VOCAB])

            # S and cq in one accumulation group: p_s = EqT.T@[Wa|Wfq] + [ba|bf]
            p_s = ps.tile([VOCAB, M + 1], F32, tag="pA", bufs=2)
            nc.tensor.matmul(p_s[:], Pa[0:DQ, _EQT:_EQT + VOCAB],
                             Pa[0:DQ, _WAQ:_WAQ + M + 1], start=True, stop=False)
            nc.tensor.matmul(p_s[:], Pb[0:1, _ONE:_ONE + VOCAB],
                             Pb[0:1, _BIA:_BIA + M + 1],
                             start=False, stop=True)
            nc.scalar.copy(mcat[0:VOCAB, 52:53], p_s[:, M:M + 1])
            # logits are O(few): exp is safe in f32 without max-subtraction
            smx = sb.tile([VOCAB, M + 2], F32)
            nc.scalar.activation(smx[:, 0:M], p_s[:, 0:M],
                                 mybir.ActivationFunctionType.Exp,
                                 bias=0.0, scale=1.0)
            nc.vector.tensor_reduce(smx[:, M + 1:M + 2], smx[:, 0:M], AX.X, OP.add)
            nc.vector.reciprocal(smx[:, M + 1:M + 2], smx[:, M + 1:M + 2])
            nc.vector.tensor_scalar(out=mcat[0:VOCAB, 0:M], in0=smx[:, 0:M],
                                    scalar1=smx[:, M + 1:M + 2], scalar2=None,
                                    op0=OP.mult)

            # ---- main pipeline ----
            osl = sb.tile([128, BS], F32)

            for pi in range(NPASS):
                oh_g = oh_t[pi]
                ws = wk.tile([128, WS], F16, tag="ws")
                pAs = []
                for half in range(2):
                    pA = ps.tile([128, 1024], F32, tag="pA", name=f"pA_{half}",
                                 bufs=2)
                    for k in range(16):
                        kb = half * 16 + k
                        nc.tensor.matmul(pA[:, k * 64:k * 64 + MC],
                                         oh_g[:, kb * T:(kb + 1) * T],
                                         mcat[:], start=True, stop=True)
                    pAs.append(pA)
                    # A -> m-major [m, b]; digit block -> c-major [c, b]
                    pAck = pA[:].rearrange("p (k c) -> p c k", c=64)
                    nc.scalar.copy(
                        ws[:, _A:_A + 1024].rearrange(
                            "p (c k) -> p c k", k=GB)[:, :, half * 16:half * 16 + 16],
                        pAck[:, 0:M, :])
                    nc.scalar.copy(
                        ws[:, _DGC:_DGC + 704].rearrange(
                            "p (c k) -> p c k", k=GB)[:, :, half * 16:half * 16 + 16],
                        pAck[:, 32:54, :])
                # cv[w] = sum_j cvt[j] * ind[j]  (c-major: flat 2D ops)
                nc.vector.tensor_tensor(
                    ws[:, _CVP:_CVP + 320], ws[:, _DGC:_DGC + 320],
                    ws[:, _DGC + 320:_DGC + 640], OP.mult)
                with nc.allow_low_precision(reason="10-term f16 dot of one-hot"):
                    nc.vector.tensor_reduce(
                        ws[:, _CVW:_CVW + GB],
                        ws[:, _CVP:_CVP + 320].rearrange(
                            "p (c k) -> p k c", k=GB),
                        AX.X, OP.add)
                # v = A * cv[w] (m-major: cv broadcast on the OUTER dim)
                a3 = ws[:, _A:_A + 1024].rearrange("p (c k) -> p c k", k=GB)
                cvb = ws[:, _CVW:_CVW + GB].rearrange("p (c k) -> p c k", c=1)
                a3b, cvb = bass.broadcast_tensor_aps(a3, cvb)
                nc.vector.tensor_tensor(
                    ws[:, _V:_V + 1024].rearrange("p (c k) -> p c k", k=GB),
                    a3b, cvb, OP.mult)
                # exclusive cumsum over t; halves split by m (columns are
                # independent), C for m 0:16 copied to f16, m 16:32 read
                # from PSUM by the second product
                pPs = []
                for half in range(2):
                    pP = ps.tile([128, 512], F32, tag="pP2", name=f"pP_{half}",
                                 bufs=2)
                    nc.tensor.matmul(pP[:], us_t,
                                     ws[:, _V + half * 512:_V + (half + 1) * 512],
                                     start=True, stop=True)
                    pPs.append(pP)
                nc.scalar.copy(ws[:, _CG:_CG + 512], pPs[0][:])
                # pred contribution: A * C then log-fold over m, all flat 2D
                nc.vector.tensor_tensor(
                    ws[:, _AP:_AP + 512], ws[:, _A:_A + 512],
                    ws[:, _CG:_CG + 512], OP.mult)
                nc.vector.tensor_tensor(
                    ws[:, _AP + 512:_AP + 1024], ws[:, _A + 512:_A + 1024],
                    pPs[1][:], OP.mult)
                with nc.allow_low_precision(reason="32-term f16 dot, tol 2e-2"):
                    nc.vector.tensor_add(ws[:, _F1:_F1 + 512],
                                         ws[:, _AP:_AP + 512],
                                         ws[:, _AP + 512:_AP + 1024])
                    nc.vector.tensor_add(ws[:, _F2:_F2 + 256],
                                         ws[:, _F1:_F1 + 256],
                                         ws[:, _F1 + 256:_F1 + 512])
                    nc.vector.tensor_add(ws[:, _F3:_F3 + 128],
                                         ws[:, _F2:_F2 + 128],
                                         ws[:, _F2 + 128:_F2 + 256])
                    nc.vector.tensor_add(ws[:, _F4:_F4 + 64],
                                         ws[:, _F3:_F3 + 64],
                                         ws[:, _F3 + 64:_F3 + 128])
                    nc.vector.tensor_add(ws[:, _O16:_O16 + GB],
                                         ws[:, _F4:_F4 + GB],
                                         ws[:, _F4 + GB:_F4 + 2 * GB])
                nc.vector.tensor_add(
                    osl[:, pi * GB:(pi + 1) * GB],
                    ws[:, _O16:_O16 + GB],
                    ws[:, _DGC + 640:_DGC + 640 + GB])
                nc.sync.dma_start(preds[:, pi * GB:(pi + 1) * GB],
                                  osl[:, pi * GB:(pi + 1) * GB])

    nc.compile()
    return nc


@functools.lru_cache(maxsize=1)
def _get_nc():
    return _build()


def _in_maps(questions, answers, Eq, Ev, Wa, ba, Wf, bf):
    questions = np.asarray(questions)
    answers = np.asarray(answers)
    w = (questions.astype(np.int64) * 2 + answers.astype(np.int64)) % VOCAB
    pack_a = np.zeros((DQ, PCA), np.float16)
    pack_a[:, _EQT:_EQT + VOCAB] = np.asarray(Eq, np.float32).T
    # Ev rows permuted so the derived cv row is emitted in (i-major) order
    perm = np.array([10 * (k % 10) + k // 10 for k in range(VOCAB)])
    pack_a[:, _EVT:_EVT + VOCAB] = np.asarray(Ev, np.float32)[perm].T
    wf = np.asarray(Wf, np.float32).reshape(DQ + DV)
    pack_a[:, _WAQ:_WAQ + M] = np.asarray(Wa, np.float32)
    pack_a[:, _WAQ + M] = wf[0:DQ]
    pack_a[:, _WFR] = wf[DQ:DQ + DV]
    pack_b = np.zeros((128, PCB), np.float16)
    pack_b[0, _BIA:_BIA + M] = np.asarray(ba, np.float32).reshape(M)
    pack_b[0, _BIA + M] = np.asarray(bf, np.float32).reshape(())
    pack_b[0, _ONE:_ONE + VOCAB] = 1.0
    pack_b[:, _US:_US + 128] = np.triu(np.ones((128, 128), np.float16), k=1)
    # mcat skeleton: zeros except I10 at rows 110:120, cols 42:52
    skel = np.zeros((R, MC), np.float16)
    skel[110:120, 42:52] = np.eye(10, dtype=np.float16)
    in_maps = []
    for c in range(NCORES):
        sl = slice(c * BS, (c + 1) * BS)
        qf = np.ascontiguousarray(questions[:, sl].T).ravel()
        wfl = np.ascontiguousarray(w[:, sl].T).ravel()
        oh = np.zeros((R, N), dtype=ml_dtypes.float8_e4m3)
        ar = np.arange(N)
        oh[qf, ar] = 1.0
        oh[100 + wfl % 10, ar] = 1.0
        oh[110 + wfl // 10, ar] = 1.0
        # contiguous per pass-chunk: [NPASS*R, GB*T]
        ohc = np.ascontiguousarray(
            oh.reshape(R, NPASS, GB * T).transpose(1, 0, 2)).reshape(
                NPASS * R, GB * T)
        in_maps.append({"pack_a": pack_a, "pack_b": pack_b, "skel": skel,
                        "ohall": ohc})
    return in_maps


def kernel(questions, answers, Eq, Ev, Wa, ba, Wf, bf):
    nc = _get_nc()
    in_maps = _in_maps(questions, answers, Eq, Ev, Wa, ba, Wf, bf)
    res = run_bass_kernel_spmd(nc, in_maps, list(range(NCORES)))
    preds = np.concatenate([res.results[c]["preds"] for c in range(NCORES)], axis=1)
    return preds.astype(np.float32)


# revision 29
# speedup vs baseline: 1.0069x; 1.0069x over previous
"""DKVMN scatter_memory kernel for 8 Trainium2 NeuronCores.

Math: the reference scan only ever uses the (B, M, Dv) memory through
read @ Wf_r, so the whole recurrence collapses to a 32-dim linear
cumulative sum:

  S  = softmax(Eq @ Wa + ba)            (100 x 32)  per-vocab att rows
  cq = Eq @ Wf[:64] + bf                (100,)
  cv = Ev @ Wf[64:]                     (100,)
  w  = (2q + a) % 100
  pred[t,b] = cq[q[t,b]] + sum_{s<t} cv[w[s,b]] * <S[q[t,b]], S[q[s,b]]>

Per core (batch-sharded, Bs=128): the host precomputes a 120-row fp8
index encoding per token (pure index preprocessing; 0/1 exact in fp8):
rows 0:100 one-hot(q), rows 100:110 one-hot(w%10), rows 110:120
one-hot(w//10).  One 54-col matmul per batch element against a packed
table mcat = [S | cvt | ind | cq | pad] gathers the S-row, cq, and the
digit factors of cv[w] (cv[w] = sum_j 1{hi=j} * cv[10j+lo]).  The
cumsum over t is a strict-upper-triangular matmul.  Layout: t on
partitions; A/V/C/A*C are stored M-MAJOR ([m, b] in the free dim) so
the cv broadcast is outer-stride-0 (keeps DVE 2x mode), the A*C
product is flat 2D, and the reduce over m is a log-fold of flat adds.
All elementwise work is on DVE (GpSimd contends with DVE for the SBUF
port); Scalar does the PSUM->SBUF extractions.
"""
import functools
import numpy as np
import ml_dtypes

import concourse.bass as bass
import concourse.bacc as bacc
import concourse.mybir as mybir
from concourse import tile
from concourse.bass_utils import run_bass_kernel_spmd

T, B, M, DQ, DV, VOCAB = 128, 1024, 32, 64, 64, 100
NCORES = 8
BS = B // NCORES  # 128
N = T * BS        # tokens per core = 16384
R = 120           # one-hot rows: 100 q + 10 w-lo + 10 w-hi
GB = 32           # b per pass
NPASS = BS // GB  # 4
F32 = mybir.dt.float32
F16 = mybir.dt.float16
FP8 = mybir.dt.float8e4
AX = mybir.AxisListType
OP = mybir.AluOpType

# mcat column layout (53 used cols at stride 54):
#   0:32  S row      32:42 cvt (cv candidates given lo digit)
#   42:52 ind (1{hi=j})   52 cq   53 pad(0)
MC = 54

# pack_a (f16 [64, PCA]): matmul params, lands first
_EQT, _EVT, _WAQ, _WFR = 0, 100, 200, 233
PCA = 234
# pack_b (f16 [128, PCB]): bias row, ones row, strict-upper US
_BIA, _ONE, _US = 0, 34, 134
PCB = 262

# merged per-pass workspace (f16), column offsets
_A = 0          # [32m, 32b] gathered S rows, m-major
_DGC = 1024     # [22c, 32b] digit block, c-major (cvt 0:10 | ind | cq @20)
_CVP = 1728     # [10c, 32b] cvt * ind, c-major
_CVW = 2048     # [32b]      cv[w]
_V = 2080       # [32m, 32b] A * cv[w], m-major
_CG = 3104      # [16m, 32b] C for m 0:16 (f16; m 16:32 read from PSUM)
_AP = 3616      # [32m, 32b] A * C, m-major
_F1 = 4640      # [16m, 32b] fold scratch
_F2 = 5152      # [8m, 32b]
_F3 = 5408      # [4m, 32b]
_F4 = 5536      # [2m, 32b]
_O16 = 5600     # [32b]      reduced pred terms
WS = 5632


def _build():
    nc = bacc.Bacc("TRN2", num_devices=NCORES, debug=False, target_bir_lowering=False)
    d = {}
    d["pack_a"] = nc.dram_tensor("pack_a", [DQ, PCA], F16, kind="ExternalInput").ap()
    d["pack_b"] = nc.dram_tensor("pack_b", [128, PCB], F16, kind="ExternalInput").ap()
    d["skel"] = nc.dram_tensor("skel", [R, MC], F16, kind="ExternalInput").ap()
    # one-hot, stored so each pass-chunk [R, GB*T] is a contiguous block
    d["ohall"] = nc.dram_tensor("ohall", [NPASS * R, GB * T], FP8,
                                kind="ExternalInput").ap()
    preds = nc.dram_tensor("preds", [T, BS], F32, kind="ExternalOutput").ap()

    with tile.TileContext(nc) as tc:
        with (
            tc.tile_pool(name="sb", bufs=1) as sb,
            tc.tile_pool(name="oh", bufs=2) as ohp,
            tc.tile_pool(name="wk", bufs=3) as wk,
            tc.tile_pool(name="ps", bufs=2, space="PSUM") as ps,
        ):
            Pa = sb.tile([DQ, PCA], F16)
            Pb = sb.tile([128, PCB], F16)
            mcat = sb.tile([R, MC], F16)
            # pack_a first on the sync ring: the scalar ring is blocked by
            # the ACT table load early on, and pack_a gates the param chain
            nc.sync.dma_start(Pa[:], d["pack_a"][:])
            nc.scalar.dma_start(Pb[:], d["pack_b"][:])
            nc.gpsimd.dma_start(mcat[:], d["skel"][:])

            # one-hot chunks in pass order over three DMA streams (sync +
            # scalar HWDGE rings, gpsimd SWDGE), all contiguous in DRAM;
            # pool rotation (bufs=2) gates chunk i+2 on pass i.
            oh_t = []
            for ci in range(NPASS):
                t_ = ohp.tile([R, GB * T], FP8, tag="oh", name=f"oh_{ci}")
                r0 = ci * R
                nc.sync.dma_start(t_[0:40, :], d["ohall"][r0:r0 + 40, :])
                nc.scalar.dma_start(t_[40:80, :], d["ohall"][r0 + 40:r0 + 80, :])
                nc.gpsimd.dma_start(t_[80:R, :], d["ohall"][r0 + 80:r0 + R, :])
                oh_t.append(t_)

            us_t = Pb[:, _US:_US + 128]

            # ---- parameter tables (no device transposes) ----
            # cv row first: it feeds the mcat spray DMA (longest dep chain)
            p_cvr = ps.tile([1, VOCAB], F32, tag="pP2", bufs=2)
            nc.tensor.matmul(p_cvr[:], Pa[0:DV, _WFR:_WFR + 1],
                             Pa[0:DV, _EVT:_EVT + VOCAB], start=True, stop=True)
            cv_row = sb.tile([1, VOCAB], F16)
            nc.scalar.copy(cv_row[:], p_cvr[:])
            # Ev arrives row-permuted (perm(k) = 10(k%10) + k//10), so the cv
            # row comes out as cv_row[0, 10i+j] = cv[10j+i]; a plain [1,100]
            # -> [10,10] DMA spray then yields mcat[100+i, 32+j] = cv[10j+i].
            nc.scalar.dma_start(mcat[100:110, 32:42], cv_row[0:1, 0:VOCAB])

            # S and cq in one accumulation group: p_s = EqT.T@[Wa|Wfq] + [ba|bf]
            p_s = ps.tile([VOCAB, M + 1], F32, tag="pA", bufs=2)
            nc.tensor.matmul(p_s[:], Pa[0:DQ, _EQT:_EQT + VOCAB],
                             Pa[0:DQ, _WAQ:_WAQ + M + 1], start=True, stop=False)
            nc.tensor.matmul(p_s[:], Pb[0:1, _ONE:_ONE + VOCAB],
                             Pb[0:1, _BIA:_BIA + M + 1],
                             start=False, stop=True)
            nc.scalar.copy(mcat[0:VOCAB, 52:53], p_s[:, M:M + 1])
            smx = sb.tile([VOCAB, M + 2], F32)
            nc.vector.tensor_reduce(smx[:, M:M + 1], p_s[:, 0:M], AX.X, OP.max)
            nc.vector.tensor_scalar_mul(smx[:, M:M + 1], smx[:, M:M + 1], -1.0)
            nc.scalar.activation(smx[:, 0:M], p_s[:, 0:M],
                                 mybir.ActivationFunctionType.Exp,
                                 bias=smx[:, M:M + 1], scale=1.0)
            nc.vector.tensor_reduce(smx[:, M + 1:M + 2], smx[:, 0:M], AX.X, OP.add)
            nc.vector.reciprocal(smx[:, M + 1:M + 2], smx[:, M + 1:M + 2])
            nc.vector.tensor_scalar(out=mcat[0:VOCAB, 0:M], in0=smx[:, 0:M],
                                    scalar1=smx[:, M + 1:M + 2], scalar2=None,
                                    op0=OP.mult)

            # ---- main pipeline ----
            osl = sb.tile([128, BS], F32)

            for pi in range(NPASS):
                oh_g = oh_t[pi]
                ws = wk.tile([128, WS], F16, tag="ws")
                pAs = []
                for half in range(2):
                    pA = ps.tile([128, 1024], F32, tag="pA", name=f"pA_{half}",
                                 bufs=2)
                    for k in range(16):
                        kb = half * 16 + k
                        nc.tensor.matmul(pA[:, k * 64:k * 64 + MC],
                                         oh_g[:, kb * T:(kb + 1) * T],
                                         mcat[:], start=True, stop=True)
                    pAs.append(pA)
                    # A -> m-major [m, b]; digit block -> c-major [c, b]
                    pAck = pA[:].rearrange("p (k c) -> p c k", c=64)
                    nc.scalar.copy(
                        ws[:, _A:_A + 1024].rearrange(
                            "p (c k) -> p c k", k=GB)[:, :, half * 16:half * 16 + 16],
                        pAck[:, 0:M, :])
                    nc.scalar.copy(
                        ws[:, _DGC:_DGC + 704].rearrange(
                            "p (c k) -> p c k", k=GB)[:, :, half * 16:half * 16 + 16],
                        pAck[:, 32:54, :])
                # cv[w] = sum_j cvt[j] * ind[j]  (c-major: flat 2D ops)
                nc.vector.tensor_tensor(
                    ws[:, _CVP:_CVP + 320], ws[:, _DGC:_DGC + 320],
                    ws[:, _DGC + 320:_DGC + 640], OP.mult)
                with nc.allow_low_precision(reason="10-term f16 dot of one-hot"):
                    nc.vector.tensor_reduce(
                        ws[:, _CVW:_CVW + GB],
                        ws[:, _CVP:_CVP + 320].rearrange(
                            "p (c k) -> p k c", k=GB),
                        AX.X, OP.add)
                # v = A * cv[w] (m-major: cv broadcast on the OUTER dim)
                a3 = ws[:, _A:_A + 1024].rearrange("p (c k) -> p c k", k=GB)
                cvb = ws[:, _CVW:_CVW + GB].rearrange("p (c k) -> p c k", c=1)
                a3b, cvb = bass.broadcast_tensor_aps(a3, cvb)
                nc.vector.tensor_tensor(
                    ws[:, _V:_V + 1024].rearrange("p (c k) -> p c k", k=GB),
                    a3b, cvb, OP.mult)
                # exclusive cumsum over t; halves split by m (columns are
                # independent), C for m 0:16 copied to f16, m 16:32 read
                # from PSUM by the second product
                pPs = []
                for half in range(2):
                    pP = ps.tile([128, 512], F32, tag="pP2", name=f"pP_{half}",
                                 bufs=2)
                    nc.tensor.matmul(pP[:], us_t,
                                     ws[:, _V + half * 512:_V + (half + 1) * 512],
                                     start=True, stop=True)
                    pPs.append(pP)
                nc.scalar.copy(ws[:, _CG:_CG + 512], pPs[0][:])
                # pred contribution: A * C then log-fold over m, all flat 2D
                nc.vector.tensor_tensor(
                    ws[:, _AP:_AP + 512], ws[:, _A:_A + 512],
                    ws[:, _CG:_CG + 512], OP.mult)
                nc.vector.tensor_tensor(
                    ws[:, _AP + 512:_AP + 1024], ws[:, _A + 512:_A + 1024],
                    pPs[1][:], OP.mult)
                with nc.allow_low_precision(reason="32-term f16 dot, tol 2e-2"):
                    nc.vector.tensor_add(ws[:, _F1:_F1 + 512],
                                         ws[:, _AP:_AP + 512],
                                         ws[:, _AP + 512:_AP + 1024])
                    nc.vector.tensor_add(ws[:, _F2:_F2 + 256],
                                         ws[:, _F1:_F1 + 256],
                                         ws[:, _F1 + 256:_F1 + 512])
                    nc.vector.tensor_add(ws[:, _F3:_F3 + 128],
                                         ws[:, _F2:_F2 + 128],
                                         ws[:, _F2 + 128:_F2 + 256])
                    nc.vector.tensor_add(ws[:, _F4:_F4 + 64],
                                         ws[:, _F3:_F3 + 64],
                                         ws[:, _F3 + 64:_F3 + 128])
                    nc.vector.tensor_add(ws[:, _O16:_O16 + GB],
                                         ws[:, _F4:_F4 + GB],
                                         ws[:, _F4 + GB:_F4 + 2 * GB])
                nc.vector.tensor_add(
                    osl[:, pi * GB:(pi + 1) * GB],
                    ws[:, _O16:_O16 + GB],
                    ws[:, _DGC + 640:_DGC + 640 + GB])
                nc.sync.dma_start(preds[:, pi * GB:(pi + 1) * GB],
                                  osl[:, pi * GB:(pi + 1) * GB])

    nc.compile()
    return nc


@functools.lru_cache(maxsize=1)
def _get_nc():
    return _build()


def _in_maps(questions, answers, Eq, Ev, Wa, ba, Wf, bf):
    questions = np.asarray(questions)
    answers = np.asarray(answers)
    w = (questions.astype(np.int64) * 2 + answers.astype(np.int64)) % VOCAB
    pack_a = np.zeros((DQ, PCA), np.float16)
    pack_a[:, _EQT:_EQT + VOCAB] = np.asarray(Eq, np.float32).T
    # Ev rows permuted so the derived cv row is emitted in (i-major) order
    perm = np.array([10 * (k % 10) + k // 10 for k in range(VOCAB)])
    pack_a[:, _EVT:_EVT + VOCAB] = np.asarray(Ev, np.float32)[perm].T
    wf = np.asarray(Wf, np.float32).reshape(DQ + DV)
    pack_a[:, _WAQ:_WAQ + M] = np.asarray(Wa, np.float32)
    pack_a[:, _WAQ + M] = wf[0:DQ]
    pack_a[:, _WFR] = wf[DQ:DQ + DV]
    pack_b = np.zeros((128, PCB), np.float16)
    pack_b[0, _BIA:_BIA + M] = np.asarray(ba, np.float32).reshape(M)
    pack_b[0, _BIA + M] = np.asarray(bf, np.float32).reshape(())
    pack_b[0, _ONE:_ONE + VOCAB] = 1.0
    pack_b[:, _US:_US + 128] = np.triu(np.ones((128, 128), np.float16), k=1)
    # mcat skeleton: zeros except I10 at rows 110:120, cols 42:52
    skel = np.zeros((R, MC), np.float16)
    skel[110:120, 42:52] = np.eye(10, dtype=np.float16)
    in_maps = []
    for c in range(NCORES):
        sl = slice(c * BS, (c + 1) * BS)
        qf = np.ascontiguousarray(questions[:, sl].T).ravel()
        wfl = np.ascontiguousarray(w[:, sl].T).ravel()
        oh = np.zeros((R, N), dtype=ml_dtypes.float8_e4m3)
        ar = np.arange(N)
        oh[qf, ar] = 1.0
        oh[100 + wfl % 10, ar] = 1.0
        oh[110 + wfl // 10, ar] = 1.0
        # contiguous per pass-chunk: [NPASS*R, GB*T]
        ohc = np.ascontiguousarray(
            oh.reshape(R, NPASS, GB * T).transpose(1, 0, 2)).reshape(
                NPASS * R, GB * T)
        in_maps.append({"pack_a": pack_a, "pack_b": pack_b, "skel": skel,
                        "ohall": ohc})
    return in_maps


def kernel(questions, answers, Eq, Ev, Wa, ba, Wf, bf):
    nc = _get_nc()
    in_maps = _in_maps(questions, answers, Eq, Ev, Wa, ba, Wf, bf)
    res = run_bass_kernel_spmd(nc, in_maps, list(range(NCORES)))
    preds = np.concatenate([res.results[c]["preds"] for c in range(NCORES)], axis=1)
    return preds.astype(np.float32)


# revision 34
# speedup vs baseline: 1.0534x; 1.0462x over previous
"""DKVMN scatter_memory kernel for 8 Trainium2 NeuronCores.

Math: the reference scan only ever uses the (B, M, Dv) memory through
read @ Wf_r, so the whole recurrence collapses to a 32-dim linear
cumulative sum:

  S  = softmax(Eq @ Wa + ba)            (100 x 32)  per-vocab att rows
  cq = Eq @ Wf[:64] + bf                (100,)
  cv = Ev @ Wf[64:]                     (100,)
  w  = (2q + a) % 100
  pred[t,b] = cq[q[t,b]] + sum_{s<t} cv[w[s,b]] * <S[q[t,b]], S[q[s,b]]>

Per core (batch-sharded, Bs=128): the host precomputes a 120-row fp8
index encoding per token (pure index preprocessing; 0/1 exact in fp8):
rows 0:100 one-hot(q), rows 100:110 one-hot(w%10), rows 110:120
one-hot(w//10).  One 54-col matmul per batch element against a packed
table mcat = [S | cvt | ind | cq | pad] gathers the S-row, cq, and the
digit factors of cv[w] (cv[w] = sum_j 1{hi=j} * cv[10j+lo]).  The
cumsum over t is a strict-upper-triangular matmul.  Layout: t on
partitions; A/V/C/A*C are stored M-MAJOR ([m, b] in the free dim) so
the cv broadcast is outer-stride-0 (keeps DVE 2x mode), the A*C
product is flat 2D, and the reduce over m is a log-fold of flat adds.
All elementwise work is on DVE (GpSimd contends with DVE for the SBUF
port); Scalar does the PSUM->SBUF extractions.
"""
import functools
import numpy as np
import ml_dtypes

import concourse.bass as bass
import concourse.bacc as bacc
import concourse.mybir as mybir
from concourse import tile
from concourse.bass_utils import run_bass_kernel_spmd

T, B, M, DQ, DV, VOCAB = 128, 1024, 32, 64, 64, 100
NCORES = 8
BS = B // NCORES  # 128
N = T * BS        # tokens per core = 16384
R = 120           # one-hot rows: 100 q + 10 w-lo + 10 w-hi
GB = 32           # b per pass
NPASS = BS // GB  # 4
F32 = mybir.dt.float32
F16 = mybir.dt.float16
FP8 = mybir.dt.float8e4
AX = mybir.AxisListType
OP = mybir.AluOpType

# mcat column layout (53 used cols at stride 54):
#   0:32  S row      32:42 cvt (cv candidates given lo digit)
#   42:52 ind (1{hi=j})   52 cq   53 pad(0)
MC = 54

# pack_a (f16 [64, PCA]): matmul params, lands first
_EQT, _EVT, _WAQ, _WFR = 0, 100, 200, 233
PCA = 234
# pack_b (f16 [128, PCB]): bias row, ones row, strict-upper US
_BIA, _ONE, _US = 0, 34, 134
PCB = 262

# merged per-pass workspace (f16), column offsets
_A = 0          # [32m, 32b] gathered S rows, m-major
_DGC = 1024     # [22c, 32b] digit block, c-major (cvt 0:10 | ind | cq @20)
_CVP = 1728     # [10c, 32b] cvt * ind, c-major
_CVW = 2048     # [32b]      cv[w]
_V = 2080       # [32m, 32b] A * cv[w], m-major
_CG = 3104      # [16m, 32b] C for m 0:16 (f16; m 16:32 read from PSUM)
_AP = 3616      # [32m, 32b] A * C, m-major
_F1 = 4640      # [16m, 32b] fold scratch
_F2 = 5152      # [8m, 32b]
_F3 = 5408      # [4m, 32b]
_F4 = 5536      # [2m, 32b]
_O16 = 5600     # [32b]      reduced pred terms
WS = 5632


def _build():
    nc = bacc.Bacc("TRN2", num_devices=NCORES, debug=False, target_bir_lowering=False)
    d = {}
    d["pack_a"] = nc.dram_tensor("pack_a", [DQ, PCA], F16, kind="ExternalInput").ap()
    d["pack_b"] = nc.dram_tensor("pack_b", [128, PCB], F16, kind="ExternalInput").ap()
    d["skel"] = nc.dram_tensor("skel", [R, MC], F16, kind="ExternalInput").ap()
    # one-hot, stored so each pass-chunk [R, GB*T] is a contiguous block
    d["ohall"] = nc.dram_tensor("ohall", [NPASS * R, GB * T], FP8,
                                kind="ExternalInput").ap()
    preds = nc.dram_tensor("preds", [T, BS], F32, kind="ExternalOutput").ap()

    with tile.TileContext(nc) as tc:
        with (
            tc.tile_pool(name="sb", bufs=1) as sb,
            tc.tile_pool(name="oh", bufs=2) as ohp,
            tc.tile_pool(name="wk", bufs=3) as wk,
            tc.tile_pool(name="ps", bufs=2, space="PSUM") as ps,
        ):
            Pa = sb.tile([DQ, PCA], F16)
            Pb = sb.tile([128, PCB], F16)
            mcat = sb.tile([R, MC], F16)
            # pack_a first on the sync ring: the scalar ring is blocked by
            # the ACT table load early on, and pack_a gates the param chain
            nc.sync.dma_start(Pa[:], d["pack_a"][:])
            nc.scalar.dma_start(Pb[:], d["pack_b"][:])
            nc.gpsimd.dma_start(mcat[:], d["skel"][:])

            # one-hot chunks in pass order over three DMA streams (sync +
            # scalar HWDGE rings, gpsimd SWDGE), all contiguous in DRAM;
            # pool rotation (bufs=2) gates chunk i+2 on pass i.
            oh_t = []
            for ci in range(NPASS):
                t_ = ohp.tile([R, GB * T], FP8, tag="oh", name=f"oh_{ci}")
                r0 = ci * R
                nc.sync.dma_start(t_[0:80, :], d["ohall"][r0:r0 + 80, :])
                nc.gpsimd.dma_start(t_[80:R, :], d["ohall"][r0 + 80:r0 + R, :])
                oh_t.append(t_)

            us_t = Pb[:, _US:_US + 128]

            # ---- parameter tables (no device transposes) ----
            # cv row first: it feeds the mcat spray DMA (longest dep chain)
            p_cvr = ps.tile([1, VOCAB], F32, tag="pP2", bufs=2)
            nc.tensor.matmul(p_cvr[:], Pa[0:DV, _WFR:_WFR + 1],
                             Pa[0:DV, _EVT:_EVT + VOCAB], start=True, stop=True)
            cv_row = sb.tile([1, VOCAB], F16)
            nc.scalar.copy(cv_row[:], p_cvr[:])
            # Ev arrives row-permuted (perm(k) = 10(k%10) + k//10), so the cv
            # row comes out as cv_row[0, 10i+j] = cv[10j+i]; a plain [1,100]
            # -> [10,10] DMA spray then yields mcat[100+i, 32+j] = cv[10j+i].
            # The scalar HWDGE ring carries only pack_b before this, so the
            # spray transfers immediately; the sync ring is FIFO-busy with
            # ~1MB of one-hot chunks and would stall it until ~14us.
            nc.scalar.dma_start(mcat[100:110, 32:42], cv_row[0:1, 0:VOCAB])

            # S and cq in one accumulation group: p_s = EqT.T@[Wa|Wfq] + [ba|bf]
            p_s = ps.tile([VOCAB, M + 1], F32, tag="pA", bufs=2)
            nc.tensor.matmul(p_s[:], Pa[0:DQ, _EQT:_EQT + VOCAB],
                             Pa[0:DQ, _WAQ:_WAQ + M + 1], start=True, stop=False)
            nc.tensor.matmul(p_s[:], Pb[0:1, _ONE:_ONE + VOCAB],
                             Pb[0:1, _BIA:_BIA + M + 1],
                             start=False, stop=True)
            nc.scalar.copy(mcat[0:VOCAB, 52:53], p_s[:, M:M + 1])
            smx = sb.tile([VOCAB, M + 2], F32)
            nc.vector.tensor_reduce(smx[:, M:M + 1], p_s[:, 0:M], AX.X, OP.max,
                                    negate=True)
            nc.scalar.activation(smx[:, 0:M], p_s[:, 0:M],
                                 mybir.ActivationFunctionType.Exp,
                                 bias=smx[:, M:M + 1], scale=1.0)
            nc.vector.tensor_reduce(smx[:, M + 1:M + 2], smx[:, 0:M], AX.X, OP.add)
            nc.vector.reciprocal(smx[:, M + 1:M + 2], smx[:, M + 1:M + 2])
            nc.vector.tensor_scalar(out=mcat[0:VOCAB, 0:M], in0=smx[:, 0:M],
                                    scalar1=smx[:, M + 1:M + 2], scalar2=None,
                                    op0=OP.mult)

            # ---- main pipeline ----
            osl = sb.tile([128, BS], F32)

            for pi in range(NPASS):
                oh_g = oh_t[pi]
                ws = wk.tile([128, WS], F16, tag="ws")
                pAs = []
                for half in range(2):
                    pA = ps.tile([128, 1024], F32, tag="pA", name=f"pA_{half}",
                                 bufs=2)
                    for k in range(16):
                        kb = half * 16 + k
                        nc.tensor.matmul(pA[:, k * 64:k * 64 + MC],
                                         oh_g[:, kb * T:(kb + 1) * T],
                                         mcat[:], start=True, stop=True)
                    pAs.append(pA)
                    # A -> m-major [m, b]; digit block -> c-major [c, b]
                    pAck = pA[:].rearrange("p (k c) -> p c k", c=64)
                    nc.scalar.copy(
                        ws[:, _A:_A + 1024].rearrange(
                            "p (c k) -> p c k", k=GB)[:, :, half * 16:half * 16 + 16],
                        pAck[:, 0:M, :])
                    nc.scalar.copy(
                        ws[:, _DGC:_DGC + 704].rearrange(
                            "p (c k) -> p c k", k=GB)[:, :, half * 16:half * 16 + 16],
                        pAck[:, 32:54, :])
                # cv[w] = sum_j cvt[j] * ind[j]  (c-major: flat 2D ops)
                nc.vector.tensor_tensor(
                    ws[:, _CVP:_CVP + 320], ws[:, _DGC:_DGC + 320],
                    ws[:, _DGC + 320:_DGC + 640], OP.mult)
                with nc.allow_low_precision(reason="10-term f16 dot of one-hot"):
                    nc.vector.tensor_reduce(
                        ws[:, _CVW:_CVW + GB],
                        ws[:, _CVP:_CVP + 320].rearrange(
                            "p (c k) -> p k c", k=GB),
                        AX.X, OP.add)
                # v = A * cv[w] (m-major: cv broadcast on the OUTER dim)
                a3 = ws[:, _A:_A + 1024].rearrange("p (c k) -> p c k", k=GB)
                cvb = ws[:, _CVW:_CVW + GB].rearrange("p (c k) -> p c k", c=1)
                a3b, cvb = bass.broadcast_tensor_aps(a3, cvb)
                nc.vector.tensor_tensor(
                    ws[:, _V:_V + 1024].rearrange("p (c k) -> p c k", k=GB),
                    a3b, cvb, OP.mult)
                # exclusive cumsum over t; halves split by m (columns are
                # independent), C for m 0:16 copied to f16, m 16:32 read
                # from PSUM by the second product
                pPs = []
                for half in range(2):
                    pP = ps.tile([128, 512], F32, tag="pP2", name=f"pP_{half}",
                                 bufs=2)
                    nc.tensor.matmul(pP[:], us_t,
                                     ws[:, _V + half * 512:_V + (half + 1) * 512],
                                     start=True, stop=True)
                    pPs.append(pP)
                nc.scalar.copy(ws[:, _CG:_CG + 512], pPs[0][:])
                # pred contribution: A * C then log-fold over m, all flat 2D
                nc.vector.tensor_tensor(
                    ws[:, _AP:_AP + 512], ws[:, _A:_A + 512],
                    ws[:, _CG:_CG + 512], OP.mult)
                nc.vector.tensor_tensor(
                    ws[:, _AP + 512:_AP + 1024], ws[:, _A + 512:_A + 1024],
                    pPs[1][:], OP.mult)
                with nc.allow_low_precision(reason="32-term f16 dot, tol 2e-2"):
                    nc.vector.tensor_add(ws[:, _F1:_F1 + 512],
                                         ws[:, _AP:_AP + 512],
                                         ws[:, _AP + 512:_AP + 1024])
                    nc.vector.tensor_add(ws[:, _F2:_F2 + 256],
                                         ws[:, _F1:_F1 + 256],
                                         ws[:, _F1 + 256:_F1 + 512])
                    # tail of the fold as one strided reduce over the 8
                    # remaining m (small flat adds are fixed-cost bound)
                    nc.vector.tensor_reduce(
                        ws[:, _O16:_O16 + GB],
                        ws[:, _F2:_F2 + 256].rearrange("p (c k) -> p k c", k=GB),
                        AX.X, OP.add)
                nc.vector.tensor_add(
                    osl[:, pi * GB:(pi + 1) * GB],
                    ws[:, _O16:_O16 + GB],
                    ws[:, _DGC + 640:_DGC + 640 + GB])
                nc.sync.dma_start(preds[:, pi * GB:(pi + 1) * GB],
                                  osl[:, pi * GB:(pi + 1) * GB])

    nc.compile()
    return nc


@functools.lru_cache(maxsize=1)
def _get_nc():
    return _build()


def _in_maps(questions, answers, Eq, Ev, Wa, ba, Wf, bf):
    questions = np.asarray(questions)
    answers = np.asarray(answers)
    w = (questions.astype(np.int64) * 2 + answers.astype(np.int64)) % VOCAB
    pack_a = np.zeros((DQ, PCA), np.float16)
    pack_a[:, _EQT:_EQT + VOCAB] = np.asarray(Eq, np.float32).T
    # Ev rows permuted so the derived cv row is emitted in (i-major) order
    perm = np.array([10 * (k % 10) + k // 10 for k in range(VOCAB)])
    pack_a[:, _EVT:_EVT + VOCAB] = np.asarray(Ev, np.float32)[perm].T
    wf = np.asarray(Wf, np.float32).reshape(DQ + DV)
    pack_a[:, _WAQ:_WAQ + M] = np.asarray(Wa, np.float32)
    pack_a[:, _WAQ + M] = wf[0:DQ]
    pack_a[:, _WFR] = wf[DQ:DQ + DV]
    pack_b = np.zeros((128, PCB), np.float16)
    pack_b[0, _BIA:_BIA + M] = np.asarray(ba, np.float32).reshape(M)
    pack_b[0, _BIA + M] = np.asarray(bf, np.float32).reshape(())
    pack_b[0, _ONE:_ONE + VOCAB] = 1.0
    pack_b[:, _US:_US + 128] = np.triu(np.ones((128, 128), np.float16), k=1)
    # mcat skeleton: zeros except I10 at rows 110:120, cols 42:52
    skel = np.zeros((R, MC), np.float16)
    skel[110:120, 42:52] = np.eye(10, dtype=np.float16)
    in_maps = []
    for c in range(NCORES):
        sl = slice(c * BS, (c + 1) * BS)
        qf = np.ascontiguousarray(questions[:, sl].T).ravel()
        wfl = np.ascontiguousarray(w[:, sl].T).ravel()
        oh = np.zeros((R, N), dtype=ml_dtypes.float8_e4m3)
        ar = np.arange(N)
        oh[qf, ar] = 1.0
        oh[100 + wfl % 10, ar] = 1.0
        oh[110 + wfl // 10, ar] = 1.0
        # contiguous per pass-chunk: [NPASS*R, GB*T]
        ohc = np.ascontiguousarray(
            oh.reshape(R, NPASS, GB * T).transpose(1, 0, 2)).reshape(
                NPASS * R, GB * T)
        in_maps.append({"pack_a": pack_a, "pack_b": pack_b, "skel": skel,
                        "ohall": ohc})
    return in_maps


def kernel(questions, answers, Eq, Ev, Wa, ba, Wf, bf):
    nc = _get_nc()
    in_maps = _in_maps(questions, answers, Eq, Ev, Wa, ba, Wf, bf)
    res = run_bass_kernel_spmd(nc, in_maps, list(range(NCORES)))
    preds = np.concatenate([res.results[c]["preds"] for c in range(NCORES)], axis=1)
    return preds.astype(np.float32)
